# revision 10
# baseline (speedup 1.0000x reference)
"""Cross-attention kernel for Trainium2, 8 NeuronCores (SPMD, no collectives).

Reference computation (per batch b of 4, 16 heads, d_head 64):
    q = x @ Wq + bq ; k = y @ Wk + bk ; v = y @ Wv + bv
    out = concat_h softmax(q_h k_h^T / 8) v_h @ Wo + bo
Shapes: x [4, 4096, 1024], y [4, 1024, 768] -> out [4, 4096, 1024].

Sharding: batch x query-rows. Core c handles batch c//2, query rows
(c%2)*2048:(c%2+1)*2048. Weights replicated. Each core's output slice is
disjoint -> no cross-core communication.

Device-side layout (everything transposed so TensorE contracts on the
partition dim; host pre-transposes x and y):
    Q^T [1024, 2048] = Wq^T x^T     (f32r matmuls, bf16 result)
    K^T [1024, 1024] = Wk^T y^T     (bf16 result)
    V~  [1024, 16*65]                (bf16, per head 64 V cols + ones col)
    per head, per q-block of 512:
      S^T [keys, q]   = K_h^T^T Q_h^T            (bf16 in, f32 psum)
      P^T = exp(S^T / 8)                          (no max-sub: |s| < ~3)
      [O^T_h; D] [65, q] = V~_h^T P^T             (ones col -> D = row sums)
      O^T_h normalized by 1/D (gpsimd partition-broadcast + DVE mul)
    Out^T [1024, 2048] = Wo^T O^T + b'            (f32r)
Bias folding: bk drops (softmax shift-invariance), bq folds into Q,
b' = bv @ Wo + bo is applied at the end (host-precomputed).
"""
import os
import sys

sys.path.insert(0, "/opt/trn_rl_repo")

# The TRN cores are reached through the jax "axon" PJRT backend; a
# JAX_PLATFORMS=cpu pin (common when running the jax reference) would hide
# them from this process.
if os.environ.get("JAX_PLATFORMS") and "axon" not in os.environ["JAX_PLATFORMS"]:
    os.environ.pop("JAX_PLATFORMS", None)

import numpy as np
import concourse.bass as bass
import concourse.mybir as mybir
import concourse.tile as tile
from concourse import bacc
from concourse import bass_utils

F32 = mybir.dt.float32
F32R = mybir.dt.float32r
BF16 = mybir.dt.bfloat16

B = 4
SQ = 4096
SKV = 1024
D_EMBED = 1024
D_CROSS = 768
NH = 16
DH = 64
NCORES = 8

RQ = SQ * B // NCORES      # 2048 query rows per core
QB = 512                   # q-block size
NBLK = RQ // QB            # 4 blocks
NPT = D_EMBED // 128       # 8 partition tiles of the embed dim
NCT = D_CROSS // 128       # 6 partition tiles of the cross dim
NKT = SKV // 128           # 8 key chunks
VW = DH + 1                # 65: V columns per head incl. ones column

_NC_CACHE = None


def _build():
    nc = bacc.Bacc("TRN2", target_bir_lowering=False, debug=False,
                   num_devices=NCORES)
    d_xT = nc.dram_tensor("xT", [D_EMBED, RQ], F32, kind="ExternalInput").ap()
    d_yT = nc.dram_tensor("yT", [D_CROSS, SKV], F32, kind="ExternalInput").ap()
    d_Wq = nc.dram_tensor("Wq", [D_EMBED, D_EMBED], F32, kind="ExternalInput").ap()
    d_Wk = nc.dram_tensor("Wk", [D_CROSS, D_EMBED], F32, kind="ExternalInput").ap()
    d_Wv = nc.dram_tensor("Wv", [D_CROSS, D_EMBED], F32, kind="ExternalInput").ap()
    d_Wo = nc.dram_tensor("Wo", [D_EMBED, D_EMBED], F32, kind="ExternalInput").ap()
    d_bq = nc.dram_tensor("bqt", [128, NPT], F32, kind="ExternalInput").ap()
    d_bf = nc.dram_tensor("bft", [128, NPT], F32, kind="ExternalInput").ap()
    d_out = nc.dram_tensor("outT", [D_EMBED, RQ], F32, kind="ExternalOutput").ap()

    Exp = mybir.ActivationFunctionType.Exp

    with tile.TileContext(nc) as tc:
        with tc.tile_pool(name="resid", bufs=1) as resid, \
             tc.tile_pool(name="psS", bufs=2, space="PSUM") as psS_pool, \
             tc.tile_pool(name="psAV", bufs=2, space="PSUM") as psAV_pool, \
             tc.tile_pool(name="psP", bufs=2, space="PSUM") as psP_pool:

            t_bq = resid.tile([128, NPT], F32, name="bq")
            nc.sync.dma_start(t_bq[:], d_bq)
            t_bf = resid.tile([128, NPT], F32, name="bf")
            nc.sync.dma_start(t_bf[:], d_bf)

            t_KT = [resid.tile([128, SKV], BF16, name=f"KT{p}") for p in range(NPT)]
            t_V = [resid.tile([128, NH * VW + DH], BF16, name=f"V{kp}") for kp in range(NKT)]

            # ---------- stage 0: K^T and V~ from y ----------
            with tc.tile_pool(name="stage0", bufs=1) as s0:
                # half-width tiles: the first projection matmuls only wait on
                # 512-col DMA pieces instead of whole 1024-col tiles
                t_yT = [[s0.tile([128, 512], F32R, name=f"yT{c}_{hf}")
                         for hf in range(2)] for c in range(NCT)]
                t_Wk = [[s0.tile([128, 512], F32R, name=f"Wk{c}_{hf}")
                         for hf in range(2)] for c in range(NCT)]
                t_Wv = [[s0.tile([128, 512], F32R, name=f"Wv{c}_{hf}")
                         for hf in range(2)] for c in range(NCT)]
                for c in range(NCT):
                    for hf in range(2):
                        nc.sync.dma_start(
                            t_yT[c][hf][:],
                            d_yT[c * 128:(c + 1) * 128, hf * 512:(hf + 1) * 512].bitcast(F32R))
                        nc.sync.dma_start(
                            t_Wk[c][hf][:],
                            d_Wk[c * 128:(c + 1) * 128, hf * 512:(hf + 1) * 512].bitcast(F32R))
                        nc.sync.dma_start(
                            t_Wv[c][hf][:],
                            d_Wv[c * 128:(c + 1) * 128, hf * 512:(hf + 1) * 512].bitcast(F32R))

                # K^T [dk, keys]
                for p in range(NPT):
                    for n in range(SKV // 512):
                        ps = psP_pool.tile([128, 512], F32, name=f"psK{p}_{n}", tag="pp")
                        for c in range(NCT):
                            nc.tensor.matmul(
                                ps[:],
                                t_Wk[c][p // 4][:, (p % 4) * 128:(p % 4 + 1) * 128],
                                t_yT[c][n][:],
                                start=(c == 0), stop=(c == NCT - 1))
                        nc.vector.tensor_copy(t_KT[p][:, n * 512:(n + 1) * 512], ps[:])

                # V~ [keys, interleaved heads]
                for kp in range(NKT):
                    for n in range(D_EMBED // 512):
                        ps = psP_pool.tile([128, 512], F32, name=f"psV{kp}_{n}", tag="pp")
                        for c in range(NCT):
                            nc.tensor.matmul(
                                ps[:],
                                t_yT[c][kp // 4][:, (kp % 4) * 128:(kp % 4 + 1) * 128],
                                t_Wv[c][n][:],
                                start=(c == 0), stop=(c == NCT - 1))
                        # scatter the 8 heads of this 512-chunk into stride-65 slots
                        dst = t_V[kp][:, n * 8 * VW:(n + 1) * 8 * VW] \
                            .rearrange("p (h c) -> p h c", h=8)[:, :, 0:DH]
                        src = ps[:].rearrange("p (h c) -> p h c", h=8)
                        nc.vector.tensor_copy(dst, src)
                    ones_ap = t_V[kp][:, 0:NH * VW].rearrange("p (h c) -> p h c", h=NH)[:, :, DH:DH + 1]
                    nc.vector.memset(ones_ap, 1.0)
                    nc.vector.memset(t_V[kp][:, NH * VW:NH * VW + DH], 0.0)

            # ---------- resident weights for the main loop ----------
            t_Wq = [resid.tile([128, D_EMBED], F32R, name=f"Wq{c}") for c in range(NPT)]
            t_Wo = [resid.tile([128, D_EMBED], F32R, name=f"Wo{c}") for c in range(NPT)]
            for c in range(NPT):
                nc.sync.dma_start(t_Wq[c][:], d_Wq[c * 128:(c + 1) * 128, :].bitcast(F32R))
                nc.sync.dma_start(t_Wo[c][:], d_Wo[c * 128:(c + 1) * 128, :].bitcast(F32R))

            with tc.tile_pool(name="xTp", bufs=12) as xT_pool, \
                 tc.tile_pool(name="QTp", bufs=12) as QT_pool, \
                 tc.tile_pool(name="PTp", bufs=10) as PT_pool, \
                 tc.tile_pool(name="OTp", bufs=14) as OT_pool, \
                 tc.tile_pool(name="outp", bufs=4) as out_pool, \
                 tc.tile_pool(name="smallp", bufs=3) as small_pool:

                all_xT = {}
                for blk in range(NBLK):
                    q0 = blk * QB
                    all_xT[blk] = [xT_pool.tile([128, QB], F32R,
                                                name=f"xT{blk}_{c}", tag="xT")
                                   for c in range(NPT)]
                    for c in range(NPT):
                        nc.sync.dma_start(
                            all_xT[blk][c][:],
                            d_xT[c * 128:(c + 1) * 128, q0:q0 + QB].bitcast(F32R))

                for blk in range(NBLK):
                    q0 = blk * QB
                    t_xT = all_xT[blk]
                    t_QT = [QT_pool.tile([128, QB], BF16, name=f"QT{blk}_{p}", tag="QT")
                            for p in range(NPT)]
                    for p in range(NPT):
                        ps = psP_pool.tile([128, QB], F32, name=f"psQ{blk}_{p}", tag="pp")
                        for c in range(NPT):
                            nc.tensor.matmul(ps[:], t_Wq[c][:, p * 128:(p + 1) * 128],
                                             t_xT[c][:], start=(c == 0), stop=(c == NPT - 1))
                        nc.vector.tensor_scalar_add(t_QT[p][:], ps[:], t_bq[:, p:p + 1])

                    t_OT = [OT_pool.tile([128, QB], F32R, name=f"OT{blk}_{p}", tag="OT")
                            for p in range(NPT)]

                    # -- heads --
                    for h in range(NH):
                        pt = h // 2
                        off = (h % 2) * DH
                        # 4 double-wide score/exp tiles, 2 key-chunks each
                        t_PT = [PT_pool.tile([128, 2 * QB], BF16,
                                             name=f"PT{blk}_{h}_{g}", tag="PT")
                                for g in range(NKT // 2)]
                        for g in range(NKT // 2):
                            ps_s = psS_pool.tile([128, 2 * QB], F32,
                                                 name=f"psS{blk}_{h}_{g}", tag="psS")
                            for half in range(2):
                                kp = 2 * g + half
                                nc.tensor.matmul(
                                    ps_s[:, half * QB:(half + 1) * QB],
                                    t_KT[pt][off:off + DH, kp * 128:(kp + 1) * 128],
                                    t_QT[pt][off:off + DH, :],
                                    start=True, stop=True)
                            nc.scalar.activation(t_PT[g][:], ps_s[:], Exp, scale=0.125)
                        ps_av = psAV_pool.tile([128, QB], F32,
                                               name=f"psAV{blk}_{h}", tag="psAV")
                        for kp in range(NKT):
                            nc.tensor.matmul(
                                ps_av[:, :],
                                t_V[kp][:, h * VW:h * VW + 128],
                                t_PT[kp // 2][:, (kp % 2) * QB:(kp % 2 + 1) * QB],
                                start=(kp == 0), stop=(kp == NKT - 1))
                        t_R = small_pool.tile([1, QB], F32, name=f"R{blk}_{h}", tag="R")
                        nc.vector.tensor_copy(t_R[:], ps_av[DH:DH + 1, :])
                        nc.vector.reciprocal_approx_fast(t_R[:], t_R[:])
                        t_Rb = small_pool.tile([64, QB], F32, name=f"Rb{blk}_{h}", tag="Rb")
                        nc.gpsimd.partition_broadcast(t_Rb[:], t_R[0:1, :], channels=64)
                        nc.vector.tensor_mul(t_OT[pt][off:off + DH, :],
                                             ps_av[0:DH, :], t_Rb[:])

                    # -- out projection --
                    for p in range(NPT):
                        ps = psP_pool.tile([128, QB], F32, name=f"psO{blk}_{p}", tag="pp")
                        for c in range(NPT):
                            nc.tensor.matmul(ps[:], t_Wo[c][:, p * 128:(p + 1) * 128],
                                             t_OT[c][:], start=(c == 0), stop=(c == NPT - 1))
                        t_out = out_pool.tile([128, QB], F32, name=f"out{blk}_{p}", tag="out")
                        nc.vector.tensor_scalar_add(t_out[:], ps[:], t_bf[:, p:p + 1])
                        nc.sync.dma_start(d_out[p * 128:(p + 1) * 128, q0:q0 + QB],
                                          t_out[:])

    nc.compile()
    return nc


def _get_nc():
    global _NC_CACHE
    if _NC_CACHE is None:
        _NC_CACHE = _build()
    return _NC_CACHE


def kernel(x, y, Wq, bq, Wk, bk, Wv, bv, Wo, bo):
    x = np.asarray(x, dtype=np.float32)
    y = np.asarray(y, dtype=np.float32)
    Wq = np.asarray(Wq, dtype=np.float32)
    Wk = np.asarray(Wk, dtype=np.float32)
    Wv = np.asarray(Wv, dtype=np.float32)
    Wo = np.asarray(Wo, dtype=np.float32)
    bq = np.asarray(bq, dtype=np.float32)
    bk = np.asarray(bk, dtype=np.float32)  # drops out of softmax
    bv = np.asarray(bv, dtype=np.float32)
    bo = np.asarray(bo, dtype=np.float32)

    bfinal = bv @ Wo + bo
    bqt = np.ascontiguousarray(bq.reshape(NPT, 128).T)
    bft = np.ascontiguousarray(bfinal.reshape(NPT, 128).T)

    nc = _get_nc()
    in_maps = []
    for c in range(NCORES):
        b = c // 2
        r0 = (c % 2) * RQ
        in_maps.append({
            "xT": np.ascontiguousarray(x[b, r0:r0 + RQ, :].T),
            "yT": np.ascontiguousarray(y[b].T),
            "Wq": Wq, "Wk": Wk, "Wv": Wv, "Wo": Wo,
            "bqt": bqt, "bft": bft,
        })

    res = bass_utils.run_bass_kernel_spmd(nc, in_maps, core_ids=list(range(NCORES)))

    out = np.empty((B, SQ, D_EMBED), dtype=np.float32)
    for c in range(NCORES):
        b = c // 2
        r0 = (c % 2) * RQ
        out[b, r0:r0 + RQ, :] = res.results[c]["outT"].T
    return out


# revision 12
# speedup vs baseline: 1.0037x; 1.0037x over previous
"""Cross-attention kernel for Trainium2, 8 NeuronCores (SPMD, no collectives).

Reference computation (per batch b of 4, 16 heads, d_head 64):
    q = x @ Wq + bq ; k = y @ Wk + bk ; v = y @ Wv + bv
    out = concat_h softmax(q_h k_h^T / 8) v_h @ Wo + bo
Shapes: x [4, 4096, 1024], y [4, 1024, 768] -> out [4, 4096, 1024].

Sharding: batch x query-rows. Core c handles batch c//2, query rows
(c%2)*2048:(c%2+1)*2048. Weights replicated. Each core's output slice is
disjoint -> no cross-core communication.

Device-side layout (everything transposed so TensorE contracts on the
partition dim; host pre-transposes x and y):
    Q^T [1024, 2048] = Wq^T x^T     (f32r matmuls, bf16 result)
    K^T [1024, 1024] = Wk^T y^T     (bf16 result)
    V~  [1024, 16*65]                (bf16, per head 64 V cols + ones col)
    per head, per q-block of 512:
      S^T [keys, q]   = K_h^T^T Q_h^T            (bf16 in, f32 psum)
      P^T = exp(S^T / 8)                          (no max-sub: |s| < ~3)
      [O^T_h; D] [65, q] = V~_h^T P^T             (ones col -> D = row sums)
      O^T_h normalized by 1/D (gpsimd partition-broadcast + DVE mul)
    Out^T [1024, 2048] = Wo^T O^T + b'            (f32r)
Bias folding: bk drops (softmax shift-invariance), bq folds into Q,
b' = bv @ Wo + bo is applied at the end (host-precomputed).
"""
import os
import sys

sys.path.insert(0, "/opt/trn_rl_repo")

# The TRN cores are reached through the jax "axon" PJRT backend; a
# JAX_PLATFORMS=cpu pin (common when running the jax reference) would hide
# them from this process.
if os.environ.get("JAX_PLATFORMS") and "axon" not in os.environ["JAX_PLATFORMS"]:
    os.environ.pop("JAX_PLATFORMS", None)

import numpy as np
import concourse.bass as bass
import concourse.mybir as mybir
import concourse.tile as tile
from concourse import bacc
from concourse import bass_utils

F32 = mybir.dt.float32
F32R = mybir.dt.float32r
BF16 = mybir.dt.bfloat16

B = 4
SQ = 4096
SKV = 1024
D_EMBED = 1024
D_CROSS = 768
NH = 16
DH = 64
NCORES = 8

RQ = SQ * B // NCORES      # 2048 query rows per core
QB = 512                   # q-block size
NBLK = RQ // QB            # 4 blocks
NPT = D_EMBED // 128       # 8 partition tiles of the embed dim
NCT = D_CROSS // 128       # 6 partition tiles of the cross dim
NKT = SKV // 128           # 8 key chunks
VW = DH + 1                # 65: V columns per head incl. ones column

_NC_CACHE = None


def _build():
    nc = bacc.Bacc("TRN2", target_bir_lowering=False, debug=False,
                   num_devices=NCORES)
    d_xT = nc.dram_tensor("xT", [D_EMBED, RQ], F32, kind="ExternalInput").ap()
    d_yT = nc.dram_tensor("yT", [D_CROSS, SKV], F32, kind="ExternalInput").ap()
    d_Wq = nc.dram_tensor("Wq", [D_EMBED, D_EMBED], F32, kind="ExternalInput").ap()
    d_Wk = nc.dram_tensor("Wk", [D_CROSS, D_EMBED], F32, kind="ExternalInput").ap()
    d_Wv = nc.dram_tensor("Wv", [D_CROSS, D_EMBED], F32, kind="ExternalInput").ap()
    d_Wo = nc.dram_tensor("Wo", [D_EMBED, D_EMBED], F32, kind="ExternalInput").ap()
    d_bq = nc.dram_tensor("bqt", [128, NPT], F32, kind="ExternalInput").ap()
    d_bf = nc.dram_tensor("bft", [128, NPT], F32, kind="ExternalInput").ap()
    d_out = nc.dram_tensor("outT", [D_EMBED, RQ], F32, kind="ExternalOutput").ap()

    Exp = mybir.ActivationFunctionType.Exp

    with tile.TileContext(nc) as tc:
        with tc.tile_pool(name="resid", bufs=1) as resid, \
             tc.tile_pool(name="psS", bufs=2, space="PSUM") as psS_pool, \
             tc.tile_pool(name="psAV", bufs=2, space="PSUM") as psAV_pool, \
             tc.tile_pool(name="psP", bufs=2, space="PSUM") as psP_pool:

            t_bq = resid.tile([128, NPT], F32, name="bq")
            nc.sync.dma_start(t_bq[:], d_bq)
            t_bf = resid.tile([128, NPT], F32, name="bf")
            nc.sync.dma_start(t_bf[:], d_bf)

            t_KT = [resid.tile([128, SKV], BF16, name=f"KT{p}") for p in range(NPT)]
            t_V = [resid.tile([128, NH * VW + DH], BF16, name=f"V{kp}") for kp in range(NKT)]

            # ---------- stage 0: K^T and V~ from y ----------
            with tc.tile_pool(name="stage0", bufs=1) as s0:
                # half-width tiles: the first projection matmuls only wait on
                # 512-col DMA pieces instead of whole 1024-col tiles
                t_yT = [[s0.tile([128, 512], F32R, name=f"yT{c}_{hf}")
                         for hf in range(2)] for c in range(NCT)]
                t_Wk = [[s0.tile([128, 512], F32R, name=f"Wk{c}_{hf}")
                         for hf in range(2)] for c in range(NCT)]
                t_Wv = [[s0.tile([128, 512], F32R, name=f"Wv{c}_{hf}")
                         for hf in range(2)] for c in range(NCT)]
                for c in range(NCT):
                    for hf in range(2):
                        nc.sync.dma_start(
                            t_yT[c][hf][:],
                            d_yT[c * 128:(c + 1) * 128, hf * 512:(hf + 1) * 512].bitcast(F32R))
                        nc.sync.dma_start(
                            t_Wk[c][hf][:],
                            d_Wk[c * 128:(c + 1) * 128, hf * 512:(hf + 1) * 512].bitcast(F32R))
                        nc.sync.dma_start(
                            t_Wv[c][hf][:],
                            d_Wv[c * 128:(c + 1) * 128, hf * 512:(hf + 1) * 512].bitcast(F32R))

                # K^T [dk, keys]
                for p in range(NPT):
                    for n in range(SKV // 512):
                        ps = psP_pool.tile([128, 512], F32, name=f"psK{p}_{n}", tag="pp")
                        for c in range(NCT):
                            nc.tensor.matmul(
                                ps[:],
                                t_Wk[c][p // 4][:, (p % 4) * 128:(p % 4 + 1) * 128],
                                t_yT[c][n][:],
                                start=(c == 0), stop=(c == NCT - 1))
                        nc.vector.tensor_copy(t_KT[p][:, n * 512:(n + 1) * 512], ps[:])

                # V~ [keys, interleaved heads]
                for kp in range(NKT):
                    for n in range(D_EMBED // 512):
                        ps = psP_pool.tile([128, 512], F32, name=f"psV{kp}_{n}", tag="pp")
                        for c in range(NCT):
                            nc.tensor.matmul(
                                ps[:],
                                t_yT[c][kp // 4][:, (kp % 4) * 128:(kp % 4 + 1) * 128],
                                t_Wv[c][n][:],
                                start=(c == 0), stop=(c == NCT - 1))
                        # scatter the 8 heads of this 512-chunk into stride-65 slots
                        dst = t_V[kp][:, n * 8 * VW:(n + 1) * 8 * VW] \
                            .rearrange("p (h c) -> p h c", h=8)[:, :, 0:DH]
                        src = ps[:].rearrange("p (h c) -> p h c", h=8)
                        nc.vector.tensor_copy(dst, src)
                    ones_ap = t_V[kp][:, 0:NH * VW].rearrange("p (h c) -> p h c", h=NH)[:, :, DH:DH + 1]
                    nc.vector.memset(ones_ap, 1.0)
                    nc.vector.memset(t_V[kp][:, NH * VW:NH * VW + DH], 0.0)

            # ---------- resident weights for the main loop ----------
            t_Wq = [resid.tile([128, D_EMBED], F32R, name=f"Wq{c}") for c in range(NPT)]
            t_Wo = [resid.tile([128, D_EMBED], F32R, name=f"Wo{c}") for c in range(NPT)]
            for c in range(NPT):
                nc.sync.dma_start(t_Wq[c][:], d_Wq[c * 128:(c + 1) * 128, :].bitcast(F32R))
                nc.sync.dma_start(t_Wo[c][:], d_Wo[c * 128:(c + 1) * 128, :].bitcast(F32R))

            with tc.tile_pool(name="xTp", bufs=12) as xT_pool, \
                 tc.tile_pool(name="QTp", bufs=12) as QT_pool, \
                 tc.tile_pool(name="PTp", bufs=10) as PT_pool, \
                 tc.tile_pool(name="OTp", bufs=14) as OT_pool, \
                 tc.tile_pool(name="outp", bufs=4) as out_pool, \
                 tc.tile_pool(name="smallp", bufs=3) as small_pool:

                all_xT = {}
                for blk in range(NBLK):
                    q0 = blk * QB
                    all_xT[blk] = [xT_pool.tile([128, QB], F32R,
                                                name=f"xT{blk}_{c}", tag="xT")
                                   for c in range(NPT)]
                    for c in range(NPT):
                        nc.sync.dma_start(
                            all_xT[blk][c][:],
                            d_xT[c * 128:(c + 1) * 128, q0:q0 + QB].bitcast(F32R))

                for blk in range(NBLK):
                    q0 = blk * QB
                    t_xT = all_xT[blk]
                    t_QT = [QT_pool.tile([128, QB], BF16, name=f"QT{blk}_{p}", tag="QT")
                            for p in range(NPT)]
                    for p in range(NPT):
                        ps = psP_pool.tile([128, QB], F32, name=f"psQ{blk}_{p}", tag="pp")
                        for c in range(NPT):
                            nc.tensor.matmul(ps[:], t_Wq[c][:, p * 128:(p + 1) * 128],
                                             t_xT[c][:], start=(c == 0), stop=(c == NPT - 1))
                        nc.vector.tensor_scalar_add(t_QT[p][:], ps[:], t_bq[:, p:p + 1])

                    t_OT = [OT_pool.tile([128, QB], F32R, name=f"OT{blk}_{p}", tag="OT")
                            for p in range(NPT)]

                    # -- heads --
                    for h in range(NH):
                        pt = h // 2
                        off = (h % 2) * DH
                        # 4 double-wide score/exp tiles, 2 key-chunks each
                        t_PT = [PT_pool.tile([128, 2 * QB], BF16,
                                             name=f"PT{blk}_{h}_{g}", tag="PT")
                                for g in range(NKT // 2)]
                        for g in range(NKT // 2):
                            ps_s = psS_pool.tile([128, 2 * QB], F32,
                                                 name=f"psS{blk}_{h}_{g}", tag="psS")
                            for half in range(2):
                                kp = 2 * g + half
                                nc.tensor.matmul(
                                    ps_s[:, half * QB:(half + 1) * QB],
                                    t_KT[pt][off:off + DH, kp * 128:(kp + 1) * 128],
                                    t_QT[pt][off:off + DH, :],
                                    start=True, stop=True)
                            nc.scalar.activation(t_PT[g][:], ps_s[:], Exp, scale=0.125)
                        ps_av = psAV_pool.tile([128, QB], F32,
                                               name=f"psAV{blk}_{h}", tag="psAV")
                        for kp in range(NKT):
                            nc.tensor.matmul(
                                ps_av[:, :],
                                t_V[kp][:, h * VW:h * VW + 128],
                                t_PT[kp // 2][:, (kp % 2) * QB:(kp % 2 + 1) * QB],
                                start=(kp == 0), stop=(kp == NKT - 1))
                        t_R = small_pool.tile([1, QB], F32, name=f"R{blk}_{h}", tag="R")
                        nc.vector.tensor_copy(t_R[:], ps_av[DH:DH + 1, :])
                        nc.vector.reciprocal_approx_fast(t_R[:], t_R[:])
                        t_Rb = small_pool.tile([64, QB], F32, name=f"Rb{blk}_{h}", tag="Rb")
                        nc.gpsimd.partition_broadcast(t_Rb[:], t_R[0:1, :], channels=64)
                        nc.vector.tensor_mul(t_OT[pt][off:off + DH, :],
                                             ps_av[0:DH, :], t_Rb[:])

                    # -- out projection --
                    for p in range(NPT):
                        ps = psP_pool.tile([128, QB], F32, name=f"psO{blk}_{p}", tag="pp")
                        for c in range(NPT):
                            nc.tensor.matmul(ps[:], t_Wo[c][:, p * 128:(p + 1) * 128],
                                             t_OT[c][:], start=(c == 0), stop=(c == NPT - 1))
                        t_out = out_pool.tile([128, QB], F32, name=f"out{blk}_{p}", tag="out")
                        nc.scalar.activation(t_out[:], ps[:],
                                             mybir.ActivationFunctionType.Identity,
                                             bias=t_bf[:, p:p + 1])
                        nc.sync.dma_start(d_out[p * 128:(p + 1) * 128, q0:q0 + QB],
                                          t_out[:])

    nc.compile()
    return nc


def _get_nc():
    global _NC_CACHE
    if _NC_CACHE is None:
        _NC_CACHE = _build()
    return _NC_CACHE


def kernel(x, y, Wq, bq, Wk, bk, Wv, bv, Wo, bo):
    x = np.asarray(x, dtype=np.float32)
    y = np.asarray(y, dtype=np.float32)
    Wq = np.asarray(Wq, dtype=np.float32)
    Wk = np.asarray(Wk, dtype=np.float32)
    Wv = np.asarray(Wv, dtype=np.float32)
    Wo = np.asarray(Wo, dtype=np.float32)
    bq = np.asarray(bq, dtype=np.float32)
    bk = np.asarray(bk, dtype=np.float32)  # drops out of softmax
    bv = np.asarray(bv, dtype=np.float32)
    bo = np.asarray(bo, dtype=np.float32)

    bfinal = bv @ Wo + bo
    bqt = np.ascontiguousarray(bq.reshape(NPT, 128).T)
    bft = np.ascontiguousarray(bfinal.reshape(NPT, 128).T)

    nc = _get_nc()
    in_maps = []
    for c in range(NCORES):
        b = c // 2
        r0 = (c % 2) * RQ
        in_maps.append({
            "xT": np.ascontiguousarray(x[b, r0:r0 + RQ, :].T),
            "yT": np.ascontiguousarray(y[b].T),
            "Wq": Wq, "Wk": Wk, "Wv": Wv, "Wo": Wo,
            "bqt": bqt, "bft": bft,
        })

    res = bass_utils.run_bass_kernel_spmd(nc, in_maps, core_ids=list(range(NCORES)))

    out = np.empty((B, SQ, D_EMBED), dtype=np.float32)
    for c in range(NCORES):
        b = c // 2
        r0 = (c % 2) * RQ
        out[b, r0:r0 + RQ, :] = res.results[c]["outT"].T
    return out


# revision 13
# speedup vs baseline: 1.0326x; 1.0288x over previous
"""Cross-attention kernel for Trainium2, 8 NeuronCores (SPMD, no collectives).

Reference computation (per batch b of 4, 16 heads, d_head 64):
    q = x @ Wq + bq ; k = y @ Wk + bk ; v = y @ Wv + bv
    out = concat_h softmax(q_h k_h^T / 8) v_h @ Wo + bo
Shapes: x [4, 4096, 1024], y [4, 1024, 768] -> out [4, 4096, 1024].

Sharding: batch x query-rows. Core c handles batch c//2, query rows
(c%2)*2048:(c%2+1)*2048. Weights replicated. Each core's output slice is
disjoint -> no cross-core communication.

Device-side layout (everything transposed so TensorE contracts on the
partition dim; host pre-transposes x and y):
    Q^T [1024, 2048] = Wq^T x^T     (f32r matmuls, bf16 result)
    K^T [1024, 1024] = Wk^T y^T     (bf16 result)
    V~  [1024, 16*65]                (bf16, per head 64 V cols + ones col)
    per head, per q-block of 512:
      S^T [keys, q]   = K_h^T^T Q_h^T            (bf16 in, f32 psum)
      P^T = exp(S^T / 8)                          (no max-sub: |s| < ~3)
      [O^T_h; D] [65, q] = V~_h^T P^T             (ones col -> D = row sums)
      O^T_h normalized by 1/D (gpsimd partition-broadcast + DVE mul)
    Out^T [1024, 2048] = Wo^T O^T + b'            (f32r)
Bias folding: bk drops (softmax shift-invariance), bq folds into Q,
b' = bv @ Wo + bo is applied at the end (host-precomputed).
"""
import os
import sys

sys.path.insert(0, "/opt/trn_rl_repo")

# The TRN cores are reached through the jax "axon" PJRT backend; a
# JAX_PLATFORMS=cpu pin (common when running the jax reference) would hide
# them from this process.
if os.environ.get("JAX_PLATFORMS") and "axon" not in os.environ["JAX_PLATFORMS"]:
    os.environ.pop("JAX_PLATFORMS", None)

import numpy as np
import concourse.bass as bass
import concourse.mybir as mybir
import concourse.tile as tile
from concourse import bacc
from concourse import bass_utils

F32 = mybir.dt.float32
F32R = mybir.dt.float32r
BF16 = mybir.dt.bfloat16

B = 4
SQ = 4096
SKV = 1024
D_EMBED = 1024
D_CROSS = 768
NH = 16
DH = 64
NCORES = 8

RQ = SQ * B // NCORES      # 2048 query rows per core
QB = 512                   # q-block size
NBLK = RQ // QB            # 4 blocks
NPT = D_EMBED // 128       # 8 partition tiles of the embed dim
NCT = D_CROSS // 128       # 6 partition tiles of the cross dim
NKT = SKV // 128           # 8 key chunks
VW = DH + 1                # 65: V columns per head incl. ones column

_NC_CACHE = None


def _build():
    nc = bacc.Bacc("TRN2", target_bir_lowering=False, debug=False,
                   num_devices=NCORES)
    d_xT = nc.dram_tensor("xT", [D_EMBED, RQ], F32, kind="ExternalInput").ap()
    d_yT = nc.dram_tensor("yT", [D_CROSS, SKV], F32, kind="ExternalInput").ap()
    d_Wq = nc.dram_tensor("Wq", [D_EMBED, D_EMBED], F32, kind="ExternalInput").ap()
    d_Wk = nc.dram_tensor("Wk", [D_CROSS, D_EMBED], F32, kind="ExternalInput").ap()
    d_Wv = nc.dram_tensor("Wv", [D_CROSS, D_EMBED], F32, kind="ExternalInput").ap()
    d_Wo = nc.dram_tensor("Wo", [D_EMBED, D_EMBED], F32, kind="ExternalInput").ap()
    d_bq = nc.dram_tensor("bqt", [128, NPT], F32, kind="ExternalInput").ap()
    d_bf = nc.dram_tensor("bft", [128, NPT], F32, kind="ExternalInput").ap()
    d_out = nc.dram_tensor("outT", [D_EMBED, RQ], F32, kind="ExternalOutput").ap()

    Exp = mybir.ActivationFunctionType.Exp

    with tile.TileContext(nc) as tc:
        with tc.tile_pool(name="resid", bufs=1) as resid, \
             tc.tile_pool(name="psS", bufs=2, space="PSUM") as psS_pool, \
             tc.tile_pool(name="psAV", bufs=2, space="PSUM") as psAV_pool, \
             tc.tile_pool(name="psP", bufs=2, space="PSUM") as psP_pool:

            t_bq = resid.tile([128, NPT], F32, name="bq")
            nc.sync.dma_start(t_bq[:], d_bq)
            t_bf = resid.tile([128, NPT], F32, name="bf")
            nc.sync.dma_start(t_bf[:], d_bf)

            t_KT = [resid.tile([128, SKV], BF16, name=f"KT{p}") for p in range(NPT)]
            t_V = [resid.tile([128, NH * VW + DH], BF16, name=f"V{kp}") for kp in range(NKT)]

            # ---------- stage 0: K^T and V~ from y ----------
            with tc.tile_pool(name="stage0", bufs=1) as s0:
                # half-width tiles: the first projection matmuls only wait on
                # 512-col DMA pieces instead of whole 1024-col tiles
                t_yT = [[s0.tile([128, 512], F32R, name=f"yT{c}_{hf}")
                         for hf in range(2)] for c in range(NCT)]
                t_Wk = [[s0.tile([128, 512], F32R, name=f"Wk{c}_{hf}")
                         for hf in range(2)] for c in range(NCT)]
                t_Wv = [[s0.tile([128, 512], F32R, name=f"Wv{c}_{hf}")
                         for hf in range(2)] for c in range(NCT)]
                for c in range(NCT):
                    for hf in range(2):
                        nc.sync.dma_start(
                            t_yT[c][hf][:],
                            d_yT[c * 128:(c + 1) * 128, hf * 512:(hf + 1) * 512].bitcast(F32R))
                        nc.sync.dma_start(
                            t_Wk[c][hf][:],
                            d_Wk[c * 128:(c + 1) * 128, hf * 512:(hf + 1) * 512].bitcast(F32R))
                        nc.sync.dma_start(
                            t_Wv[c][hf][:],
                            d_Wv[c * 128:(c + 1) * 128, hf * 512:(hf + 1) * 512].bitcast(F32R))

                # K^T [dk, keys]
                for p in range(NPT):
                    for n in range(SKV // 512):
                        ps = psP_pool.tile([128, 512], F32, name=f"psK{p}_{n}", tag="pp")
                        for c in range(NCT):
                            nc.tensor.matmul(
                                ps[:],
                                t_Wk[c][p // 4][:, (p % 4) * 128:(p % 4 + 1) * 128],
                                t_yT[c][n][:],
                                start=(c == 0), stop=(c == NCT - 1))
                        nc.vector.tensor_copy(t_KT[p][:, n * 512:(n + 1) * 512], ps[:])

                # V~ [keys, interleaved heads]
                for kp in range(NKT):
                    for n in range(D_EMBED // 512):
                        ps = psP_pool.tile([128, 512], F32, name=f"psV{kp}_{n}", tag="pp")
                        for c in range(NCT):
                            nc.tensor.matmul(
                                ps[:],
                                t_yT[c][kp // 4][:, (kp % 4) * 128:(kp % 4 + 1) * 128],
                                t_Wv[c][n][:],
                                start=(c == 0), stop=(c == NCT - 1))
                        # scatter the 8 heads of this 512-chunk into stride-65 slots
                        dst = t_V[kp][:, n * 8 * VW:(n + 1) * 8 * VW] \
                            .rearrange("p (h c) -> p h c", h=8)[:, :, 0:DH]
                        src = ps[:].rearrange("p (h c) -> p h c", h=8)
                        nc.vector.tensor_copy(dst, src)
                    ones_ap = t_V[kp][:, 0:NH * VW].rearrange("p (h c) -> p h c", h=NH)[:, :, DH:DH + 1]
                    nc.vector.memset(ones_ap, 1.0)
                    nc.vector.memset(t_V[kp][:, NH * VW:NH * VW + DH], 0.0)

            # ---------- resident weights for the main loop ----------
            t_Wq = [resid.tile([128, D_EMBED], F32R, name=f"Wq{c}") for c in range(NPT)]
            t_Wo = [resid.tile([128, D_EMBED], F32R, name=f"Wo{c}") for c in range(NPT)]
            for c in range(NPT):
                nc.sync.dma_start(t_Wq[c][:], d_Wq[c * 128:(c + 1) * 128, :].bitcast(F32R))
                nc.sync.dma_start(t_Wo[c][:], d_Wo[c * 128:(c + 1) * 128, :].bitcast(F32R))

            with tc.tile_pool(name="xTp", bufs=12) as xT_pool, \
                 tc.tile_pool(name="QTp", bufs=12) as QT_pool, \
                 tc.tile_pool(name="PTp", bufs=10) as PT_pool, \
                 tc.tile_pool(name="OTp", bufs=14) as OT_pool, \
                 tc.tile_pool(name="outp", bufs=4) as out_pool, \
                 tc.tile_pool(name="smallp", bufs=3) as small_pool:

                all_xT = {}
                for blk in range(NBLK):
                    q0 = blk * QB
                    all_xT[blk] = [xT_pool.tile([128, QB], F32R,
                                                name=f"xT{blk}_{c}", tag="xT")
                                   for c in range(NPT)]
                    for c in range(NPT):
                        nc.sync.dma_start(
                            all_xT[blk][c][:],
                            d_xT[c * 128:(c + 1) * 128, blk * QB:(blk + 1) * QB].bitcast(F32R))

                def emit_qproj(blk):
                    t_QT = [QT_pool.tile([128, QB], BF16, name=f"QT{blk}_{p}", tag="QT")
                            for p in range(NPT)]
                    for p in range(NPT):
                        ps = psP_pool.tile([128, QB], F32, name=f"psQ{blk}_{p}", tag="pp")
                        for c in range(NPT):
                            nc.tensor.matmul(ps[:], t_Wq[c][:, p * 128:(p + 1) * 128],
                                             all_xT[blk][c][:],
                                             start=(c == 0), stop=(c == NPT - 1))
                        nc.vector.tensor_scalar_add(t_QT[p][:], ps[:], t_bq[:, p:p + 1])
                    return t_QT

                def emit_heads(blk, t_QT):
                    t_OT = [OT_pool.tile([128, QB], F32R, name=f"OT{blk}_{p}", tag="OT")
                            for p in range(NPT)]
                    for h in range(NH):
                        pt = h // 2
                        off = (h % 2) * DH
                        t_PT = [PT_pool.tile([128, 2 * QB], BF16,
                                             name=f"PT{blk}_{h}_{g}", tag="PT")
                                for g in range(NKT // 2)]
                        for g in range(NKT // 2):
                            ps_s = psS_pool.tile([128, 2 * QB], F32,
                                                 name=f"psS{blk}_{h}_{g}", tag="psS")
                            for half in range(2):
                                kp = 2 * g + half
                                nc.tensor.matmul(
                                    ps_s[:, half * QB:(half + 1) * QB],
                                    t_KT[pt][off:off + DH, kp * 128:(kp + 1) * 128],
                                    t_QT[pt][off:off + DH, :],
                                    start=True, stop=True)
                            nc.scalar.activation(t_PT[g][:], ps_s[:], Exp, scale=0.125)
                        ps_av = psAV_pool.tile([128, QB], F32,
                                               name=f"psAV{blk}_{h}", tag="psAV")
                        for kp in range(NKT):
                            nc.tensor.matmul(
                                ps_av[:, :],
                                t_V[kp][:, h * VW:h * VW + 128],
                                t_PT[kp // 2][:, (kp % 2) * QB:(kp % 2 + 1) * QB],
                                start=(kp == 0), stop=(kp == NKT - 1))
                        t_R = small_pool.tile([1, QB], F32, name=f"R{blk}_{h}", tag="R")
                        nc.vector.tensor_copy(t_R[:], ps_av[DH:DH + 1, :])
                        nc.vector.reciprocal_approx_fast(t_R[:], t_R[:])
                        t_Rb = small_pool.tile([64, QB], F32, name=f"Rb{blk}_{h}", tag="Rb")
                        nc.gpsimd.partition_broadcast(t_Rb[:], t_R[0:1, :], channels=64)
                        nc.vector.tensor_mul(t_OT[pt][off:off + DH, :],
                                             ps_av[0:DH, :], t_Rb[:])
                    return t_OT

                def emit_outproj(blk, t_OT):
                    q0 = blk * QB
                    for p in range(NPT):
                        ps = psP_pool.tile([128, QB], F32, name=f"psO{blk}_{p}", tag="pp")
                        for c in range(NPT):
                            nc.tensor.matmul(ps[:], t_Wo[c][:, p * 128:(p + 1) * 128],
                                             t_OT[c][:], start=(c == 0), stop=(c == NPT - 1))
                        t_out = out_pool.tile([128, QB], F32, name=f"out{blk}_{p}", tag="out")
                        nc.scalar.activation(t_out[:], ps[:],
                                             mybir.ActivationFunctionType.Identity,
                                             bias=t_bf[:, p:p + 1])
                        nc.sync.dma_start(d_out[p * 128:(p + 1) * 128, q0:q0 + QB],
                                          t_out[:])

                # Q-proj for the NEXT block is emitted before this block's
                # out-proj: the shared "pp" PSUM pool hands out slots in
                # emission order, so this keeps next-block Q-proj runnable
                # while out-proj waits on the last head's normalize tail.
                t_QT = emit_qproj(0)
                for blk in range(NBLK):
                    t_OT = emit_heads(blk, t_QT)
                    if blk + 1 < NBLK:
                        t_QT = emit_qproj(blk + 1)
                    emit_outproj(blk, t_OT)

    nc.compile()
    return nc


def _get_nc():
    global _NC_CACHE
    if _NC_CACHE is None:
        _NC_CACHE = _build()
    return _NC_CACHE


def kernel(x, y, Wq, bq, Wk, bk, Wv, bv, Wo, bo):
    x = np.asarray(x, dtype=np.float32)
    y = np.asarray(y, dtype=np.float32)
    Wq = np.asarray(Wq, dtype=np.float32)
    Wk = np.asarray(Wk, dtype=np.float32)
    Wv = np.asarray(Wv, dtype=np.float32)
    Wo = np.asarray(Wo, dtype=np.float32)
    bq = np.asarray(bq, dtype=np.float32)
    bk = np.asarray(bk, dtype=np.float32)  # drops out of softmax
    bv = np.asarray(bv, dtype=np.float32)
    bo = np.asarray(bo, dtype=np.float32)

    bfinal = bv @ Wo + bo
    bqt = np.ascontiguousarray(bq.reshape(NPT, 128).T)
    bft = np.ascontiguousarray(bfinal.reshape(NPT, 128).T)

    nc = _get_nc()
    in_maps = []
    for c in range(NCORES):
        b = c // 2
        r0 = (c % 2) * RQ
        in_maps.append({
            "xT": np.ascontiguousarray(x[b, r0:r0 + RQ, :].T),
            "yT": np.ascontiguousarray(y[b].T),
            "Wq": Wq, "Wk": Wk, "Wv": Wv, "Wo": Wo,
            "bqt": bqt, "bft": bft,
        })

    res = bass_utils.run_bass_kernel_spmd(nc, in_maps, core_ids=list(range(NCORES)))

    out = np.empty((B, SQ, D_EMBED), dtype=np.float32)
    for c in range(NCORES):
        b = c // 2
        r0 = (c % 2) * RQ
        out[b, r0:r0 + RQ, :] = res.results[c]["outT"].T
    return out
